# revision 10
# baseline (speedup 1.0000x reference)
"""Single-head attention (B=4, T=4096, E=1024, D=64) on 8 TRN2 NeuronCores.

Sharding: data-parallel over (batch, query-half): core c -> batch c//2,
query half c%2.  Each core receives the full x[b] pre-transposed on the
host, with rows rotated so its OWN query half always occupies columns
0:2048 (keeps the SPMD graph identical across cores; attention is
permutation-invariant over keys).

Row-packed scores: key tiles are paired (8q+i, 8q+4+i) within each
1024-col quarter q.  k2p [128, 2048] holds K^T for the even chunk of a
quarter in rows 0:64 and the odd chunk in rows 64:128 (projection
stationaries [Wk|Wv] / [Wv|Wk] land K on the right partition half).
q2d [128, TH] holds Q^T duplicated to both halves via a [Wq|Wq]
stationary.  The two score matmuls of a pair run CONCURRENTLY on
disjoint PE row-groups, halving score time.

Head: x^T rides the sync HWDGE ring as FIVE ordered column-block
loads (512+512+1024*3 cols, all-e packed) so the first-needed columns
complete at full aggregate DMA rate ~9us in; weights ride the scalar
ring.  Pair-0 projections are emitted chunk-major (Q c0, KV c0, Q c1,
KV c1) so the first scores launch as soon as x cols 0:1024 land.

exp split: most score chunks go through the ACT table exp; a fixed
subset (pass0 pp%4==3, pass1 pp%3==2) runs on the otherwise-idle DVE
via the bf16 bit-trick  P = bitcast_bf16(int16(s*128*log2e + bias))
(~1.8% rms multiplicative noise, zero mean in log via magic bias),
rebalancing the ACT train from ~68us to ~49us busy.

V' = [V | ones] strips: quarters 0,1 via PE transpose (interleaved
into early pass-0 slots; a DMA-transpose would deadlock-guard-wait on
the x stream), quarters 2,3 via batched DMA-transpose.  The ones
column makes P @ V' emit softmax row sums.

Per 1024-query pass, per key-tile pair: concurrent S^T matmuls, exp
(ACT or DVE) PSUM -> SBUF bf16, O^T += V'.T @ P^T into a [65, 1024]
PSUM accumulator (1/sqrt(D) folded into Wv).  AV emission lags the
score train (thr ~7 early in pass 0, ramping down) so it never blocks
the in-order PE queue on a not-yet-transposed V' strip.

Epilogue: O^T -> bf16; pass 0 via one batched DMA-transpose, last
pass via PE transposes; one reciprocal over the sum columns +
broadcast multiplies; bf16 stores in two halves on two queues (host
upcasts to f32).

PSUM: stp 3x[128,1024] (6 banks) + ot [65,1024] (2 banks) = 8 banks.

Softmax runs without max-subtraction: scores are ~N(0, 64) so |s| << 88
(fp32 exp overflow); the reference's max-subtraction is a no-op.
"""

import os
import sys
from collections import deque

import numpy as np

_TRN_REPO = "/opt/trn_rl_repo"
if _TRN_REPO not in sys.path:
    sys.path.insert(0, _TRN_REPO)

import concourse.bass as bass  # noqa: E402
import concourse.mybir as mybir  # noqa: E402
import concourse.tile as tile  # noqa: E402
from concourse import bacc  # noqa: E402
from concourse.bass_utils import run_bass_kernel_spmd  # noqa: E402

F32 = mybir.dt.float32
F16 = mybir.dt.float16
BF16 = mybir.dt.bfloat16
I16 = mybir.dt.int16

B, T, E, D = 4, 4096, 1024, 64
TH = T // 2  # queries per core
NCORES = 8
QPASS = 1024  # queries per PSUM pass
NMM = 512  # matmul moving free dim (one fp32 PSUM bank)
NKT = T // 128  # 32 key tiles of 128
EK = E // 128  # 8 contraction tiles for projections
QW = T // 4  # x^T block width (1024)

SCORE_DT = F16
SCORE_NP = np.float16
PV_DT = BF16  # P = exp(S) reaches ~1e20: needs bf16 range

# DVE bit-trick exp: P ~= bitcast_bf16(int16(s * 128*log2e + BIAS)).
# BIAS = 127*128 - 7.33 makes the piecewise-linear mantissa error
# zero-mean in log space (so DVE-exp'd keys carry no systematic weight
# offset vs ACT-exp'd keys in the same softmax row).
EXP_SCALE = 128.0 / float(np.log(2.0))
EXP_BIAS = 127.0 * 128.0 - 7.33


def _dve_chunk(qp, pp, half):
    """Which exp chunks run on DVE instead of the ACT table.  Running
    the two halves of a pp on DIFFERENT engines frees both st PSUM
    tiles concurrently (3-buf rotation would otherwise stall the score
    matmuls 1.5 slots later on the serial ACT train)."""
    if qp == 0:
        return half == 1 and pp % 2 == 1
    return half == 1


def _build_nc() -> bass.Bass:
    nc = bacc.Bacc(
        "TRN2",
        target_bir_lowering=False,
        debug=False,
        num_devices=NCORES,
    )
    xT_d = nc.dram_tensor("xT", [E, T], SCORE_DT, kind="ExternalInput")
    # [Wq|Wq | Wk|Wv/8 | Wv/8|Wk] packed so ONE dma covers all weights
    w3_d = nc.dram_tensor("w3", [E, 384], SCORE_DT, kind="ExternalInput")
    out_d = nc.dram_tensor("out", [TH, D], PV_DT, kind="ExternalOutput")

    with tile.TileContext(nc) as tc:
        with (
            tc.tile_pool(name="consts", bufs=1) as consts,
            tc.tile_pool(name="big", bufs=1) as big,
            tc.tile_pool(name="pt", bufs=14) as ptpool,
            tc.tile_pool(name="osb", bufs=2) as osbpool,
            tc.tile_pool(name="small", bufs=6) as small,
            tc.tile_pool(name="stp", bufs=3, space="PSUM") as stp,
            tc.tile_pool(name="otp", bufs=1, space="PSUM") as otp,
        ):
            # ---- weights: three ordered loads on the scalar ring so
            # the Wq block (needed first) clears the ring fast and the
            # x stream on the sync ring gets full SDMA share sooner ----
            # SBUF layout: three 1024-col groups ([Wq|Wq], [Wk|Wv],
            # [Wv|Wk]), each e-major, loaded as three ordered dmas so
            # the Wq group (needed first) clears the scalar ring fast
            # and the x stream gets full SDMA share sooner.
            wtiles = []
            for b in range(3):
                wt = consts.tile([128, EK * 128], SCORE_DT, tag=f"w3_{b}")
                nc.scalar.dma_start(
                    wt[:].rearrange("p (e m) -> p e m", e=EK),
                    w3_d[:, b * 128 : (b + 1) * 128].rearrange(
                        "(e p) m -> p e m", p=128
                    ),
                )
                wtiles.append(wt)

            def wqq(e):
                return wtiles[0][:, e * 128 : e * 128 + 128]

            def wkv(e):
                return wtiles[1][:, e * 128 : e * 128 + 128]

            def wvk(e):
                return wtiles[2][:, e * 128 : e * 128 + 128]

            # ---- x^T loads: five ordered column-block transfers on the
            # sync HWDGE ring (FIFO per ring; each InstDMACopy spreads
            # across all 16 SDMA engines, so block k completes before
            # block k+1 starts).  Chunks 0,1 ride alone so the first
            # projections can start ~9us in. ----
            xb0 = []
            for h in range(2):
                xt = big.tile([128, EK * NMM], SCORE_DT, tag=f"xb0{h}")
                nc.sync.dma_start(
                    xt[:].rearrange("p (e m) -> p e m", e=EK),
                    xT_d[:, h * NMM : (h + 1) * NMM].rearrange(
                        "(e p) m -> p e m", p=128
                    ),
                )
                xb0.append(xt)
            xblk = {}
            for b in (1, 2, 3):
                xt = big.tile([128, EK * QW], SCORE_DT, tag=f"xblk{b}")
                nc.sync.dma_start(
                    xt[:].rearrange("p (e m) -> p e m", e=EK),
                    xT_d[:, b * QW : (b + 1) * QW].rearrange(
                        "(e p) m -> p e m", p=128
                    ),
                )
                xblk[b] = xt

            def xt_ap(e, cg):
                # proj chunk cg covers x^T cols [cg*512, cg*512+512)
                if cg < 2:
                    return xb0[cg][:, e * NMM : (e + 1) * NMM]
                b, half = divmod(cg, 2)
                c0 = e * QW + half * NMM
                return xblk[b][:, c0 : c0 + NMM]

            # warm tile first: the HAM warmup matmuls must start the
            # moment the preamble ends, and DVE runs its queue in order
            warm = consts.tile([128, NMM], SCORE_DT, tag="warm")
            nc.vector.memset(warm[:], 0.0)
            # V' strip: 32 tiles of [128 keys, 64 V cols + 1 ones col],
            # padded to stride 128.  Only the ones columns need init.
            vprime = consts.tile([128, NKT * 128], PV_DT, tag="vprime")
            nc.vector.memset(
                vprime[:].rearrange("p (b m) -> p b m", m=128)[:, :, 64:65],
                1.0,
            )
            ident = consts.tile([128, 64], PV_DT, tag="ident")
            from concourse.masks import make_identity

            make_identity(nc, ident[0:64, :])
            make_identity(nc, ident[64:128, :])
            ident65 = consts.tile([65, 65], PV_DT, tag="ident65")
            make_identity(nc, ident65[:])
            # preload the exp table set (~2.7us) while DMAs stream
            pre = small.tile([128, 32], PV_DT, tag="pre")
            nc.scalar.activation(
                pre[:], warm[:, 0:32], mybir.ActivationFunctionType.Exp
            )
            # HAM warmup: keep PE busy (and the pstate ramp warm) until
            # the first x block lands
            wps = stp.tile([128, QPASS], F32, tag="st", name="wps")
            for _ in range(17):
                nc.tensor.matmul(
                    wps[:, 0:256], warm[:, 0:128], warm[:, 0:256],
                    start=True, stop=True,
                )

            q2d = big.tile([128, TH], SCORE_DT, tag="q2d")
            k2p = big.tile([128, T // 2], SCORE_DT, tag="k2p")
            vt = big.tile([128, T], PV_DT, tag="vt")

            # ---- projection pair emitter: pair = chunks (cg0, cg0+1)
            # covering quarter q=cg0//2.  K of the even chunk -> k2p
            # rows 0:64, odd chunk -> rows 64:128 (via [Wk|Wv]/[Wv|Wk]);
            # V^T to vt rows 64:128 / 0:64 respectively. ----
            proj_t = {}

            def emit_proj(cg0, part, sub):
                """One 4-MM slice of a projection pair.  part 0/1 =
                [Q|Q] sweep chunk cg0/cg0+1 (own pairs only); part 2/3 =
                K/V sweep ([Wk|Wv] even chunk / [Wv|Wk] odd).  sub=1
                finishes the chunk and emits its copies (+V' transpose
                for quarters 2,3)."""
                q = cg0 // 2
                kcol = q * NMM  # k2p column block for this quarter
                kb0 = 8 * q
                half = part & 1  # 0: even chunk cg0, 1: odd chunk cg0+1
                cg = cg0 + half
                sl = slice(half * NMM, (half + 1) * NMM)

                if part in (0, 1):  # [Q|Q] sweep halves (own pairs only)
                    if part == 0 and sub == 0:
                        proj_t[cg0] = stp.tile(
                            [128, QPASS], F32, tag="st", name=f"p1_{cg0}"
                        )
                    p1 = proj_t[cg0]
                    for e in range(4 * sub, 4 * sub + 4):
                        nc.tensor.matmul(
                            p1[:, sl],
                            wqq(e),
                            xt_ap(e, cg),
                            start=(e == 0),
                            stop=(e == EK - 1),
                        )
                    if sub == 1:
                        nc.vector.tensor_copy(
                            q2d[:, cg * NMM : (cg + 1) * NMM], p1[:, sl]
                        )
                    return

                # K/V sweep: wkv for the even chunk (K -> k2p rows 0:64,
                # V^T -> vt rows 64:128), wvk for the odd (swapped)
                if part == 2 and sub == 0:
                    proj_t[cg0 + 8] = stp.tile(
                        [128, QPASS], F32, tag="st", name=f"p2_{cg0}"
                    )
                p2 = proj_t[cg0 + 8]
                w = wkv if half == 0 else wvk
                for e in range(4 * sub, 4 * sub + 4):
                    nc.tensor.matmul(
                        p2[:, sl],
                        w(e),
                        xt_ap(e, cg),
                        start=(e == 0),
                        stop=(e == EK - 1),
                    )
                if sub == 0:
                    return
                vrow, krow = (64, 0) if half == 0 else (0, 64)
                # k2p first: the score matmuls block on it; vt only
                # feeds the V' transposes, which run later
                nc.vector.tensor_copy(
                    k2p[krow : krow + 64, kcol : kcol + NMM],
                    p2[krow : krow + 64, sl],
                )
                nc.vector.tensor_copy(
                    vt[vrow : vrow + 64, cg * NMM : (cg + 1) * NMM],
                    p2[vrow : vrow + 64, sl],
                )
                if cg0 >= 4:
                    nc.sync.dma_start(
                        out=vprime[
                            :, (kb0 + 4 * half) * 128 : (kb0 + 4 * half + 4) * 128
                        ].rearrange("p (b m) -> p b m", m=128)[:, :, 0:64],
                        in_=vt[vrow : vrow + 64, cg * NMM : (cg + 1) * NMM],
                        transpose=True,
                    )

            def emit_tp0(cg):
                # V' strips for chunk cg (quarters 0,1) via PE
                # transpose: a DMA-transpose would deadlock-guard-wait
                # on the whole x stream.  All 4 strips of the chunk in
                # one PSUM tile + one batched copy (fewer st-pool
                # rotations).
                vrow = 64 if cg % 2 == 0 else 0
                tps = stp.tile(
                    [128, QPASS], PV_DT, tag="st", name=f"tp0_{cg}"
                )
                for s in range(4):
                    nc.tensor.transpose(
                        tps[:, s * 64 : (s + 1) * 64],
                        vt[
                            vrow : vrow + 64,
                            cg * NMM + s * 128 : cg * NMM + (s + 1) * 128,
                        ],
                        ident[vrow : vrow + 64, :],
                    )
                nc.vector.tensor_copy(
                    vprime[
                        :, (4 * cg) * 128 : (4 * cg + 4) * 128
                    ].rearrange("p (b m) -> p b m", m=128)[:, :, 0:64],
                    tps[:, 0:256].rearrange("p (b m) -> p b m", m=64),
                )

            # ---- pair-0 projections, chunk-major so the first scores
            # only wait on x cols 0:1024: Q c0, KV c0, Q c1, KV c1 ----
            p1_0 = stp.tile([128, QPASS], F32, tag="st", name="p1_0")
            p2_0 = stp.tile([128, QPASS], F32, tag="st", name="p2_0")
            proj_t[0] = p1_0
            proj_t[8] = p2_0
            pair0_vt = []
            for e in range(EK):
                nc.tensor.matmul(
                    p1_0[:, 0:NMM], wqq(e), xt_ap(e, 0),
                    start=(e == 0), stop=(e == EK - 1),
                )
            nc.vector.tensor_copy(q2d[:, 0:NMM], p1_0[:, 0:NMM])
            for e in range(EK):
                nc.tensor.matmul(
                    p2_0[:, 0:NMM], wkv(e), xt_ap(e, 0),
                    start=(e == 0), stop=(e == EK - 1),
                )
            nc.vector.tensor_copy(k2p[0:64, 0:NMM], p2_0[0:64, 0:NMM])
            pair0_vt.append(
                lambda: nc.vector.tensor_copy(
                    vt[64:128, 0:NMM], p2_0[64:128, 0:NMM]
                )
            )
            for e in range(EK):
                nc.tensor.matmul(
                    p1_0[:, NMM:QPASS], wqq(e), xt_ap(e, 1),
                    start=(e == 0), stop=(e == EK - 1),
                )
            nc.vector.tensor_copy(q2d[:, NMM:QPASS], p1_0[:, NMM:QPASS])
            for e in range(EK):
                nc.tensor.matmul(
                    p2_0[:, NMM:QPASS], wvk(e), xt_ap(e, 1),
                    start=(e == 0), stop=(e == EK - 1),
                )
            nc.vector.tensor_copy(k2p[64:128, 0:NMM], p2_0[64:128, NMM:QPASS])
            # vt copies go AFTER both k2p copies on the DVE queue — the
            # first scores block on k2p; the V' strips run a bit later
            for fn in pair0_vt:
                fn()
            pair0_vt.clear()
            nc.vector.tensor_copy(vt[0:64, NMM : 2 * NMM], p2_0[0:64, NMM:QPASS])
            # quarter-0 V' strips in the head: PE would otherwise idle
            # here waiting on the k2p copies before the first scores
            emit_tp0(0)
            emit_tp0(1)

            # pass-0 interleave queues (each item = one deferred
            # emission slot).  strips: V' via PE transpose for quarter
            # 1; kv1: quarter-1 KV pair; proj2: quarters 2,3 KV
            # (gated to x block arrivals); projq: pair-2 Q sweep
            # feeding pass 1's q2d.
            pending_strips = deque([(2,), (3,)])
            pending_kv1 = deque([(2, p, s) for p in (2, 3) for s in range(2)])
            pending_proj2 = deque(
                [(cg, p, s) for cg in (4, 6) for p in (2, 3) for s in range(2)]
            )
            pending_projq = deque([(2, p, s) for p in (0, 1) for s in range(2)])

            # ---- attention passes ----
            for qp in range(TH // QPASS):
                q0 = qp * QPASS
                ot = otp.tile([D + 1, QPASS], F32, tag="ot")
                pending_av = deque()

                def emit_av(avpt, avkt):
                    for qc in range(0, QPASS, NMM):
                        nc.tensor.matmul(
                            ot[:, qc : qc + NMM],
                            vprime[:, avkt * 128 : avkt * 128 + D + 1],
                            avpt[:, qc : qc + NMM],
                            start=(avkt == 0),
                            stop=(avkt == NKT - 1),
                        )

                deferred = []
                for pp in range(16):
                    # key-tile pair (8q+i, 8q+4+i): q = pp//4, i = pp%4
                    qq, i = divmod(pp, 4)
                    ktA = 8 * qq + i
                    ktB = ktA + 4
                    kc = qq * NMM + i * 128
                    sts = []
                    for half in (0, 1):
                        st = stp.tile(
                            [128, QPASS], F32, tag="st",
                            name=f"st{qp}_{pp}_{half}",
                        )
                        sts.append(st)
                    if qp == 0 and pp == 0:
                        # A-half first: the first exp unblocks earlier
                        mm_order = [(q, h) for h in (0, 1) for q in (0, NMM)]
                    else:
                        mm_order = [(q, h) for q in (0, NMM) for h in (0, 1)]
                    for qc, half in mm_order:
                        base = 64 * half
                        nc.tensor.matmul(
                            sts[half][:, qc : qc + NMM],
                            k2p[base : base + 64, kc : kc + 128],
                            q2d[base : base + 64, q0 + qc : q0 + qc + NMM],
                            start=True,
                            stop=True,
                        )
                    # previous slot's AV/projection PE work goes AFTER
                    # this slot's score matmuls: the exp train then only
                    # ever waits on the 4 ST matmuls at a block's head
                    for fn in deferred:
                        fn()
                    deferred = []
                    # AV backlog: deep early in pass 0 (V' strips and
                    # the x stream must stay ahead of the in-order PE
                    # queue), draining to 1 by the pass tail
                    thr = (
                        max(1, 7 - 2 * max(0, pp - 11))
                        if qp == 0
                        else (1 if pp == 15 else 2)
                    )
                    for half, kt in ((0, ktA), (1, ktB)):
                        pt = ptpool.tile(
                            [128, QPASS], PV_DT, tag="pt",
                            name=f"pt{qp}_{pp}_{half}",
                        )
                        if _dve_chunk(qp, pp, half):
                            nc.vector.tensor_scalar(
                                pt[:].bitcast(I16),
                                sts[half][:],
                                EXP_SCALE,
                                EXP_BIAS,
                                mybir.AluOpType.mult,
                                mybir.AluOpType.add,
                            )
                        else:
                            nc.scalar.activation(
                                pt[:], sts[half][:],
                                mybir.ActivationFunctionType.Exp,
                            )
                        pending_av.append((pt, kt))
                        while len(pending_av) > thr:
                            deferred.append(
                                (lambda a: lambda: emit_av(*a))(
                                    pending_av.popleft()
                                )
                            )
                        if qp == 0:
                            if pending_kv1:
                                deferred.append(
                                    (lambda it: lambda: emit_proj(*it))(
                                        pending_kv1.popleft()
                                    )
                                )
                            if pending_strips:
                                deferred.append(
                                    (lambda it: lambda: emit_tp0(*it))(
                                        pending_strips.popleft()
                                    )
                                )
                            elif pp >= 4 and pending_proj2:
                                deferred.append(
                                    (lambda it: lambda: emit_proj(*it))(
                                        pending_proj2.popleft()
                                    )
                                )
                            elif (
                                pp >= 10 and half == 0 and pending_projq
                            ):
                                deferred.append(
                                    (lambda it: lambda: emit_proj(*it))(
                                        pending_projq.popleft()
                                    )
                                )
                for fn in deferred:
                    fn()
                while pending_av:
                    emit_av(*pending_av.popleft())

                # epilogue: O^T -> bf16, one batched DMA-transpose (src
                # partitions %16: pad 65->80; pad rows left uninitialized
                # — their transposed columns are never read), divide by
                # the sums column, store in two halves on two queues
                last = qp == TH // QPASS - 1
                osb = osbpool.tile([80, QPASS], PV_DT, tag="osb")
                ostrip = osbpool.tile([128, QPASS // 128 * D], PV_DT, tag="ostrip")
                if last:
                    # PE transposes + division straight from PSUM, in
                    # two pipelined halves: no DMA-transpose on the tail
                    # (the xbar path costs ~1.3us + 1.5us sem wait), and
                    # half 0's div/store overlaps half 1's transposes
                    tpo = stp.tile([128, QPASS], PV_DT, tag="st", name="tpf")
                    for hf in range(2):
                        nc.vector.tensor_copy(
                            osb[0 : D + 1, hf * NMM : (hf + 1) * NMM],
                            ot[:, hf * NMM : (hf + 1) * NMM],
                        )
                        for blk in range(4 * hf, 4 * hf + 4):
                            nc.tensor.transpose(
                                tpo[:, blk * 80 : blk * 80 + D + 1],
                                osb[0 : D + 1, blk * 128 : (blk + 1) * 128],
                                ident65[:],
                            )
                        tpo_v = tpo[:, hf * 320 : (hf + 1) * 320].rearrange(
                            "p (b m) -> p b m", m=80
                        )
                        rc4 = small.tile(
                            [128, 4], F32, tag="rc4", name=f"rc4_{hf}"
                        )
                        nc.vector.reciprocal(
                            rc4[:].rearrange("p (b m) -> p b m", m=1),
                            tpo_v[:, :, D : D + 1],
                        )
                        nc.vector.tensor_tensor(
                            ostrip[:, 4 * hf * D : (4 * hf + 4) * D].rearrange(
                                "p (b d) -> p b d", d=D
                            ),
                            tpo_v[:, :, 0:D],
                            rc4[:]
                            .rearrange("p (b m) -> p b m", m=1)
                            .broadcast_to([128, 4, D]),
                            mybir.AluOpType.mult,
                        )
                        (nc.sync, nc.scalar)[hf].dma_start(
                            out_d[
                                q0 + hf * NMM : q0 + (hf + 1) * NMM, :
                            ].rearrange("(b p) d -> p b d", p=128),
                            ostrip[:, 4 * hf * D : (4 * hf + 4) * D].rearrange(
                                "p (b d) -> p b d", d=D
                            ),
                        )
                else:
                    nc.vector.tensor_copy(osb[0 : D + 1, :], ot[:])
                    tpo = osbpool.tile(
                        [128, QPASS // 128 * 80], PV_DT, tag="tpo"
                    )
                    nc.sync.dma_start(
                        out=tpo[:].rearrange("p (b m) -> p b m", m=80),
                        in_=osb[:],
                        transpose=True,
                    )
                    tpo_v = tpo[:, 0 : QPASS // 128 * 80].rearrange(
                        "p (b m) -> p b m", m=80
                    )
                    rc8 = small.tile([128, QPASS // 128], F32, tag="rc8")
                    nc.vector.reciprocal(
                        rc8[:].rearrange("p (b m) -> p b m", m=1),
                        tpo_v[:, :, D : D + 1],
                    )
                    for hf in range(2):
                        nc.vector.tensor_tensor(
                            ostrip[:, 4 * hf * D : (4 * hf + 4) * D].rearrange(
                                "p (b d) -> p b d", d=D
                            ),
                            tpo_v[:, 4 * hf : 4 * hf + 4, 0:D],
                            rc8[:, 4 * hf : 4 * hf + 4]
                            .rearrange("p (b m) -> p b m", m=1)
                            .broadcast_to([128, 4, D]),
                            mybir.AluOpType.mult,
                        )
                        (nc.sync, nc.gpsimd)[hf].dma_start(
                            out_d[
                                q0 + hf * NMM : q0 + (hf + 1) * NMM, :
                            ].rearrange("(b p) d -> p b d", p=128),
                            ostrip[:, 4 * hf * D : (4 * hf + 4) * D].rearrange(
                                "p (b d) -> p b d", d=D
                            ),
                        )

    _elide_redundant_ldweights(nc)
    nc.compile()
    return nc


def _elide_redundant_ldweights(nc):
    """Drop an InstLdweights whose stationary AP is identical to the
    previous one with only plain matmuls between (the legalizer emits one
    load per matmul; consecutive same-weights loads are dead)."""
    removed = 0
    for blk in nc.main_func.blocks:
        last_key = {}  # row-group (base partition span) -> AP key
        keep = []
        for inst in blk.instructions:
            if isinstance(inst, mybir.InstLdweights):
                si = inst.sync_info
                clean = si is None or (not si.on_wait and not si.on_update)
                ap = inst.ins[0]
                key = repr(ap)
                bap = getattr(ap, "bass_ap", None)
                part0 = psz = None
                if bap is not None:
                    try:
                        part0 = bap.base_partition()
                        psz = bap.partition_size()
                    except Exception:
                        part0 = psz = None
                grp = (part0, psz)
                full = psz is None or part0 is None or psz > 64
                if clean and part0 is not None and last_key.get(grp) == key:
                    removed += 1
                    continue
                if full:
                    last_key.clear()
                    if part0 is not None:
                        last_key[grp] = key
                else:
                    # a load into one row-group leaves other groups intact
                    last_key = {
                        g: k
                        for g, k in last_key.items()
                        if g[0] + (g[1] or 128) <= part0
                        or part0 + (psz or 128) <= g[0]
                    }
                    last_key[grp] = key
                keep.append(inst)
                continue
            if getattr(inst, "engine", None) == mybir.EngineType.PE:
                if not (
                    isinstance(inst, mybir.InstMatmult)
                    and not getattr(inst, "is_transpose", False)
                ):
                    last_key = {}
            keep.append(inst)
        blk.instructions[:] = keep
    return removed


_NC_CACHE = None
LAST_RESULT = None


def _get_nc():
    global _NC_CACHE
    if _NC_CACHE is None:
        _NC_CACHE = _build_nc()
    return _NC_CACHE


def make_in_maps(x, Wq, Wk, Wv):
    x = np.asarray(x, dtype=np.float32)
    Wq = np.asarray(Wq, dtype=np.float32)
    Wk = np.asarray(Wk, dtype=np.float32)
    Wv = np.asarray(Wv, dtype=np.float32)
    wv8 = Wv / np.sqrt(np.float32(D))
    w3 = np.ascontiguousarray(
        np.concatenate([Wq, Wq, Wk, wv8, wv8, Wk], axis=1)
    ).astype(SCORE_NP)
    in_maps = []
    for c in range(NCORES):
        b, h = divmod(c, 2)
        xb = x[b]
        rot = np.concatenate([xb[h * TH : (h + 1) * TH], xb[(1 - h) * TH : (2 - h) * TH]])
        in_maps.append(
            {
                "xT": np.ascontiguousarray(rot.T).astype(SCORE_NP),
                "w3": w3,
            }
        )
    return in_maps


def run(in_maps, trace=False, **kwargs):
    global LAST_RESULT
    nc = _get_nc()
    LAST_RESULT = run_bass_kernel_spmd(
        nc, in_maps, core_ids=list(range(NCORES)), trace=trace, **kwargs
    )
    return LAST_RESULT


def assemble(results):
    out = np.empty((B, T, D), dtype=np.float32)
    for c in range(NCORES):
        b, h = divmod(c, 2)
        out[b, h * TH : (h + 1) * TH] = np.asarray(
            results[c]["out"], dtype=np.float32
        )
    return out


def kernel(x, Wq, Wk, Wv):
    res = run(make_in_maps(x, Wq, Wk, Wv), trace=bool(os.environ.get("BASS_TRACE")))
    return assemble(res.results)


# revision 11
# speedup vs baseline: 1.0294x; 1.0294x over previous
"""Single-head attention (B=4, T=4096, E=1024, D=64) on 8 TRN2 NeuronCores.

Sharding: data-parallel over (batch, query-half): core c -> batch c//2,
query half c%2.  Each core receives the full x[b] pre-transposed on the
host, with rows rotated so its OWN query half always occupies columns
0:2048 (keeps the SPMD graph identical across cores; attention is
permutation-invariant over keys).

Row-packed scores: key tiles are paired (8q+i, 8q+4+i) within each
1024-col quarter q.  k2p [128, 2048] holds K^T for the even chunk of a
quarter in rows 0:64 and the odd chunk in rows 64:128 (projection
stationaries [Wk|Wv] / [Wv|Wk] land K on the right partition half).
q2d [128, TH] holds Q^T duplicated to both halves via a [Wq|Wq]
stationary.  The two score matmuls of a pair run CONCURRENTLY on
disjoint PE row-groups, halving score time.

Head: x^T rides the sync HWDGE ring as FIVE ordered column-block
loads (512+512+1024*3 cols, all-e packed) so the first-needed columns
complete at full aggregate DMA rate ~9us in; weights ride the scalar
ring.  Pair-0 projections are emitted chunk-major (Q c0, KV c0, Q c1,
KV c1) so the first scores launch as soon as x cols 0:1024 land.

exp split: most score chunks go through the ACT table exp; a fixed
subset (pass0 pp%4==3, pass1 pp%3==2) runs on the otherwise-idle DVE
via the bf16 bit-trick  P = bitcast_bf16(int16(s*128*log2e + bias))
(~1.8% rms multiplicative noise, zero mean in log via magic bias),
rebalancing the ACT train from ~68us to ~49us busy.

V' = [V | ones] strips: quarters 0,1 via PE transpose (interleaved
into early pass-0 slots; a DMA-transpose would deadlock-guard-wait on
the x stream), quarters 2,3 via batched DMA-transpose.  The ones
column makes P @ V' emit softmax row sums.

Per 1024-query pass, per key-tile pair: concurrent S^T matmuls, exp
(ACT or DVE) PSUM -> SBUF bf16, O^T += V'.T @ P^T into a [65, 1024]
PSUM accumulator (1/sqrt(D) folded into Wv).  AV emission lags the
score train (thr ~7 early in pass 0, ramping down) so it never blocks
the in-order PE queue on a not-yet-transposed V' strip.

Epilogue: O^T -> bf16; pass 0 via one batched DMA-transpose, last
pass via PE transposes; one reciprocal over the sum columns +
broadcast multiplies; bf16 stores in two halves on two queues (host
upcasts to f32).

PSUM: stp 3x[128,1024] (6 banks) + ot [65,1024] (2 banks) = 8 banks.

Softmax runs without max-subtraction: scores are ~N(0, 64) so |s| << 88
(fp32 exp overflow); the reference's max-subtraction is a no-op.
"""

import os
import sys
from collections import deque

import numpy as np

_TRN_REPO = "/opt/trn_rl_repo"
if _TRN_REPO not in sys.path:
    sys.path.insert(0, _TRN_REPO)

import concourse.bass as bass  # noqa: E402
import concourse.mybir as mybir  # noqa: E402
import concourse.tile as tile  # noqa: E402
from concourse import bacc  # noqa: E402
from concourse.bass_utils import run_bass_kernel_spmd  # noqa: E402

F32 = mybir.dt.float32
F16 = mybir.dt.float16
BF16 = mybir.dt.bfloat16
I16 = mybir.dt.int16

B, T, E, D = 4, 4096, 1024, 64
TH = T // 2  # queries per core
NCORES = 8
QPASS = 1024  # queries per PSUM pass
NMM = 512  # matmul moving free dim (one fp32 PSUM bank)
NKT = T // 128  # 32 key tiles of 128
EK = E // 128  # 8 contraction tiles for projections
QW = T // 4  # x^T block width (1024)

SCORE_DT = F16
SCORE_NP = np.float16
PV_DT = BF16  # P = exp(S) reaches ~1e20: needs bf16 range

# DVE bit-trick exp: P ~= bitcast_bf16(int16(s * 128*log2e + BIAS)).
# BIAS = 127*128 - 7.33 makes the piecewise-linear mantissa error
# zero-mean in log space (so DVE-exp'd keys carry no systematic weight
# offset vs ACT-exp'd keys in the same softmax row).
EXP_SCALE = 128.0 / float(np.log(2.0))
EXP_BIAS = 127.0 * 128.0 - 7.33


def _dve_chunk(qp, pp, half):
    """Which exp chunks run on DVE instead of the ACT table.  Running
    the two halves of a pp on DIFFERENT engines frees both st PSUM
    tiles concurrently (3-buf rotation would otherwise stall the score
    matmuls 1.5 slots later on the serial ACT train)."""
    if qp == 0:
        return half == 1 and pp % 2 == 1
    return half == 1


def _build_nc() -> bass.Bass:
    nc = bacc.Bacc(
        "TRN2",
        target_bir_lowering=False,
        debug=False,
        num_devices=NCORES,
    )
    xT_d = nc.dram_tensor("xT", [E, T], SCORE_DT, kind="ExternalInput")
    # [Wq|Wq | Wk|Wv/8 | Wv/8|Wk] packed so ONE dma covers all weights
    w3_d = nc.dram_tensor("w3", [E, 384], SCORE_DT, kind="ExternalInput")
    out_d = nc.dram_tensor("out", [TH, D], PV_DT, kind="ExternalOutput")

    with tile.TileContext(nc) as tc:
        with (
            tc.tile_pool(name="consts", bufs=1) as consts,
            tc.tile_pool(name="big", bufs=1) as big,
            tc.tile_pool(name="pt", bufs=14) as ptpool,
            tc.tile_pool(name="osb", bufs=2) as osbpool,
            tc.tile_pool(name="small", bufs=6) as small,
            tc.tile_pool(name="stp", bufs=3, space="PSUM") as stp,
            tc.tile_pool(name="otp", bufs=1, space="PSUM") as otp,
        ):
            # ---- weights: three ordered loads on the scalar ring so
            # the Wq block (needed first) clears the ring fast and the
            # x stream on the sync ring gets full SDMA share sooner ----
            # SBUF layout: three 1024-col groups ([Wq|Wq], [Wk|Wv],
            # [Wv|Wk]), each e-major, loaded as three ordered dmas so
            # the Wq group (needed first) clears the scalar ring fast
            # and the x stream gets full SDMA share sooner.
            # single fat load: splitting by weight block halves the
            # descriptor size to 256B (sub-line-rate) and lands slower
            w3 = consts.tile([128, EK * 384], SCORE_DT, tag="w3")
            nc.scalar.dma_start(
                w3[:].rearrange("p (e m) -> p e m", e=EK),
                w3_d.rearrange("(e p) m -> p e m", p=128),
            )

            def wqq(e):
                return w3[:, e * 384 : e * 384 + 128]

            def wkv(e):
                return w3[:, e * 384 + 128 : e * 384 + 256]

            def wvk(e):
                return w3[:, e * 384 + 256 : e * 384 + 384]

            # ---- x^T loads: five ordered column-block transfers on the
            # sync HWDGE ring (FIFO per ring; each InstDMACopy spreads
            # across all 16 SDMA engines, so block k completes before
            # block k+1 starts).  Chunks 0,1 ride alone so the first
            # projections can start ~9us in. ----
            xb0 = []
            for h in range(2):
                xt = big.tile([128, EK * NMM], SCORE_DT, tag=f"xb0{h}")
                nc.sync.dma_start(
                    xt[:].rearrange("p (e m) -> p e m", e=EK),
                    xT_d[:, h * NMM : (h + 1) * NMM].rearrange(
                        "(e p) m -> p e m", p=128
                    ),
                )
                xb0.append(xt)
            xblk = {}
            for b in (1, 2, 3):
                xt = big.tile([128, EK * QW], SCORE_DT, tag=f"xblk{b}")
                nc.sync.dma_start(
                    xt[:].rearrange("p (e m) -> p e m", e=EK),
                    xT_d[:, b * QW : (b + 1) * QW].rearrange(
                        "(e p) m -> p e m", p=128
                    ),
                )
                xblk[b] = xt

            def xt_ap(e, cg):
                # proj chunk cg covers x^T cols [cg*512, cg*512+512)
                if cg < 2:
                    return xb0[cg][:, e * NMM : (e + 1) * NMM]
                b, half = divmod(cg, 2)
                c0 = e * QW + half * NMM
                return xblk[b][:, c0 : c0 + NMM]

            # warm tile first: the HAM warmup matmuls must start the
            # moment the preamble ends, and DVE runs its queue in order
            warm = consts.tile([128, NMM], SCORE_DT, tag="warm")
            nc.vector.memset(warm[:], 0.0)
            # V' strip: 32 tiles of [128 keys, 64 V cols + 1 ones col],
            # padded to stride 128.  Only the ones columns need init.
            vprime = consts.tile([128, NKT * 128], PV_DT, tag="vprime")
            nc.vector.memset(
                vprime[:].rearrange("p (b m) -> p b m", m=128)[:, :, 64:65],
                1.0,
            )
            ident = consts.tile([128, 64], PV_DT, tag="ident")
            from concourse.masks import make_identity

            make_identity(nc, ident[0:64, :])
            make_identity(nc, ident[64:128, :])
            ident65 = consts.tile([65, 65], PV_DT, tag="ident65")
            make_identity(nc, ident65[:])
            # preload the exp table set (~2.7us) while DMAs stream
            pre = small.tile([128, 32], PV_DT, tag="pre")
            nc.scalar.activation(
                pre[:], warm[:, 0:32], mybir.ActivationFunctionType.Exp
            )
            # HAM warmup: keep PE busy (and the pstate ramp warm) until
            # the first x block lands
            wps = stp.tile([128, QPASS], F32, tag="st", name="wps")
            for _ in range(17):
                nc.tensor.matmul(
                    wps[:, 0:256], warm[:, 0:128], warm[:, 0:256],
                    start=True, stop=True,
                )

            q2d = big.tile([128, TH], SCORE_DT, tag="q2d")
            k2p = big.tile([128, T // 2], SCORE_DT, tag="k2p")
            vt = big.tile([128, T], PV_DT, tag="vt")

            # ---- projection pair emitter: pair = chunks (cg0, cg0+1)
            # covering quarter q=cg0//2.  K of the even chunk -> k2p
            # rows 0:64, odd chunk -> rows 64:128 (via [Wk|Wv]/[Wv|Wk]);
            # V^T to vt rows 64:128 / 0:64 respectively. ----
            proj_t = {}

            def emit_proj(cg0, part, sub):
                """One 4-MM slice of a projection pair.  part 0/1 =
                [Q|Q] sweep chunk cg0/cg0+1 (own pairs only); part 2/3 =
                K/V sweep ([Wk|Wv] even chunk / [Wv|Wk] odd).  sub=1
                finishes the chunk and emits its copies (+V' transpose
                for quarters 2,3)."""
                q = cg0 // 2
                kcol = q * NMM  # k2p column block for this quarter
                kb0 = 8 * q
                half = part & 1  # 0: even chunk cg0, 1: odd chunk cg0+1
                cg = cg0 + half
                sl = slice(half * NMM, (half + 1) * NMM)

                if part in (0, 1):  # [Q|Q] sweep halves (own pairs only)
                    if part == 0 and sub == 0:
                        proj_t[cg0] = stp.tile(
                            [128, QPASS], F32, tag="st", name=f"p1_{cg0}"
                        )
                    p1 = proj_t[cg0]
                    for e in range(4 * sub, 4 * sub + 4):
                        nc.tensor.matmul(
                            p1[:, sl],
                            wqq(e),
                            xt_ap(e, cg),
                            start=(e == 0),
                            stop=(e == EK - 1),
                        )
                    if sub == 1:
                        nc.vector.tensor_copy(
                            q2d[:, cg * NMM : (cg + 1) * NMM], p1[:, sl]
                        )
                    return

                # K/V sweep: wkv for the even chunk (K -> k2p rows 0:64,
                # V^T -> vt rows 64:128), wvk for the odd (swapped)
                if part == 2 and sub == 0:
                    proj_t[cg0 + 8] = stp.tile(
                        [128, QPASS], F32, tag="st", name=f"p2_{cg0}"
                    )
                p2 = proj_t[cg0 + 8]
                w = wkv if half == 0 else wvk
                for e in range(4 * sub, 4 * sub + 4):
                    nc.tensor.matmul(
                        p2[:, sl],
                        w(e),
                        xt_ap(e, cg),
                        start=(e == 0),
                        stop=(e == EK - 1),
                    )
                if sub == 0:
                    return
                vrow, krow = (64, 0) if half == 0 else (0, 64)
                # k2p first: the score matmuls block on it; vt only
                # feeds the V' transposes, which run later
                nc.vector.tensor_copy(
                    k2p[krow : krow + 64, kcol : kcol + NMM],
                    p2[krow : krow + 64, sl],
                )
                nc.vector.tensor_copy(
                    vt[vrow : vrow + 64, cg * NMM : (cg + 1) * NMM],
                    p2[vrow : vrow + 64, sl],
                )
                if cg0 >= 4:
                    nc.sync.dma_start(
                        out=vprime[
                            :, (kb0 + 4 * half) * 128 : (kb0 + 4 * half + 4) * 128
                        ].rearrange("p (b m) -> p b m", m=128)[:, :, 0:64],
                        in_=vt[vrow : vrow + 64, cg * NMM : (cg + 1) * NMM],
                        transpose=True,
                    )

            def emit_tp0(cg):
                # V' strips for chunk cg (quarters 0,1) via PE
                # transpose: a DMA-transpose would deadlock-guard-wait
                # on the whole x stream.  All 4 strips of the chunk in
                # one PSUM tile + one batched copy (fewer st-pool
                # rotations).
                vrow = 64 if cg % 2 == 0 else 0
                tps = stp.tile(
                    [128, QPASS], PV_DT, tag="st", name=f"tp0_{cg}"
                )
                for s in range(4):
                    nc.tensor.transpose(
                        tps[:, s * 64 : (s + 1) * 64],
                        vt[
                            vrow : vrow + 64,
                            cg * NMM + s * 128 : cg * NMM + (s + 1) * 128,
                        ],
                        ident[vrow : vrow + 64, :],
                    )
                nc.vector.tensor_copy(
                    vprime[
                        :, (4 * cg) * 128 : (4 * cg + 4) * 128
                    ].rearrange("p (b m) -> p b m", m=128)[:, :, 0:64],
                    tps[:, 0:256].rearrange("p (b m) -> p b m", m=64),
                )

            # ---- pair-0 projections, chunk-major so the first scores
            # only wait on x cols 0:1024: Q c0, KV c0, Q c1, KV c1 ----
            p1_0 = stp.tile([128, QPASS], F32, tag="st", name="p1_0")
            p2_0 = stp.tile([128, QPASS], F32, tag="st", name="p2_0")
            proj_t[0] = p1_0
            proj_t[8] = p2_0
            pair0_vt = []
            for e in range(EK):
                nc.tensor.matmul(
                    p1_0[:, 0:NMM], wqq(e), xt_ap(e, 0),
                    start=(e == 0), stop=(e == EK - 1),
                )
            nc.vector.tensor_copy(q2d[:, 0:NMM], p1_0[:, 0:NMM])
            for e in range(EK):
                nc.tensor.matmul(
                    p2_0[:, 0:NMM], wkv(e), xt_ap(e, 0),
                    start=(e == 0), stop=(e == EK - 1),
                )
            nc.vector.tensor_copy(k2p[0:64, 0:NMM], p2_0[0:64, 0:NMM])
            pair0_vt.append(
                lambda: nc.vector.tensor_copy(
                    vt[64:128, 0:NMM], p2_0[64:128, 0:NMM]
                )
            )
            for e in range(EK):
                nc.tensor.matmul(
                    p1_0[:, NMM:QPASS], wqq(e), xt_ap(e, 1),
                    start=(e == 0), stop=(e == EK - 1),
                )
            nc.vector.tensor_copy(q2d[:, NMM:QPASS], p1_0[:, NMM:QPASS])
            for e in range(EK):
                nc.tensor.matmul(
                    p2_0[:, NMM:QPASS], wvk(e), xt_ap(e, 1),
                    start=(e == 0), stop=(e == EK - 1),
                )
            nc.vector.tensor_copy(k2p[64:128, 0:NMM], p2_0[64:128, NMM:QPASS])
            # vt copies go AFTER both k2p copies on the DVE queue — the
            # first scores block on k2p; the V' strips run a bit later
            for fn in pair0_vt:
                fn()
            pair0_vt.clear()
            nc.vector.tensor_copy(vt[0:64, NMM : 2 * NMM], p2_0[0:64, NMM:QPASS])
            # quarter-0 V' strips in the head: PE would otherwise idle
            # here waiting on the k2p copies before the first scores
            emit_tp0(0)
            emit_tp0(1)

            # pass-0 interleave queues (each item = one deferred
            # emission slot).  strips: V' via PE transpose for quarter
            # 1; kv1: quarter-1 KV pair; proj2: quarters 2,3 KV
            # (gated to x block arrivals); projq: pair-2 Q sweep
            # feeding pass 1's q2d.
            pending_strips = deque([(2,), (3,)])
            pending_kv1 = deque([(2, p, s) for p in (2, 3) for s in range(2)])
            pending_proj2 = deque(
                [(cg, p, s) for cg in (4, 6) for p in (2, 3) for s in range(2)]
            )
            pending_projq = deque([(2, p, s) for p in (0, 1) for s in range(2)])

            # ---- attention passes ----
            for qp in range(TH // QPASS):
                q0 = qp * QPASS
                ot = otp.tile([D + 1, QPASS], F32, tag="ot")
                pending_av = deque()

                def emit_av(avpt, avkt):
                    for qc in range(0, QPASS, NMM):
                        nc.tensor.matmul(
                            ot[:, qc : qc + NMM],
                            vprime[:, avkt * 128 : avkt * 128 + D + 1],
                            avpt[:, qc : qc + NMM],
                            start=(avkt == 0),
                            stop=(avkt == NKT - 1),
                        )

                deferred = []
                for pp in range(16):
                    # key-tile pair (8q+i, 8q+4+i): q = pp//4, i = pp%4
                    qq, i = divmod(pp, 4)
                    ktA = 8 * qq + i
                    ktB = ktA + 4
                    kc = qq * NMM + i * 128
                    sts = []
                    for half in (0, 1):
                        st = stp.tile(
                            [128, QPASS], F32, tag="st",
                            name=f"st{qp}_{pp}_{half}",
                        )
                        sts.append(st)
                    if qp == 0 and pp == 0:
                        # A-half first: the first exp unblocks earlier
                        mm_order = [(q, h) for h in (0, 1) for q in (0, NMM)]
                    else:
                        mm_order = [(q, h) for q in (0, NMM) for h in (0, 1)]
                    for qc, half in mm_order:
                        base = 64 * half
                        nc.tensor.matmul(
                            sts[half][:, qc : qc + NMM],
                            k2p[base : base + 64, kc : kc + 128],
                            q2d[base : base + 64, q0 + qc : q0 + qc + NMM],
                            start=True,
                            stop=True,
                        )
                    # previous slot's AV/projection PE work goes AFTER
                    # this slot's score matmuls: the exp train then only
                    # ever waits on the 4 ST matmuls at a block's head
                    for fn in deferred:
                        fn()
                    deferred = []
                    # AV backlog: deep early in pass 0 (V' strips and
                    # the x stream must stay ahead of the in-order PE
                    # queue), draining to 1 by the pass tail
                    thr = (
                        max(1, 7 - 2 * max(0, pp - 11))
                        if qp == 0
                        else (1 if pp == 15 else 2)
                    )
                    for half, kt in ((0, ktA), (1, ktB)):
                        pt = ptpool.tile(
                            [128, QPASS], PV_DT, tag="pt",
                            name=f"pt{qp}_{pp}_{half}",
                        )
                        if _dve_chunk(qp, pp, half):
                            nc.vector.tensor_scalar(
                                pt[:].bitcast(I16),
                                sts[half][:],
                                EXP_SCALE,
                                EXP_BIAS,
                                mybir.AluOpType.mult,
                                mybir.AluOpType.add,
                            )
                        else:
                            nc.scalar.activation(
                                pt[:], sts[half][:],
                                mybir.ActivationFunctionType.Exp,
                            )
                        pending_av.append((pt, kt))
                        while len(pending_av) > thr:
                            deferred.append(
                                (lambda a: lambda: emit_av(*a))(
                                    pending_av.popleft()
                                )
                            )
                        if qp == 0:
                            if pending_kv1:
                                deferred.append(
                                    (lambda it: lambda: emit_proj(*it))(
                                        pending_kv1.popleft()
                                    )
                                )
                            if pending_strips:
                                deferred.append(
                                    (lambda it: lambda: emit_tp0(*it))(
                                        pending_strips.popleft()
                                    )
                                )
                            elif pp >= 4 and pending_proj2:
                                deferred.append(
                                    (lambda it: lambda: emit_proj(*it))(
                                        pending_proj2.popleft()
                                    )
                                )
                            elif (
                                pp >= 10 and half == 0 and pending_projq
                            ):
                                deferred.append(
                                    (lambda it: lambda: emit_proj(*it))(
                                        pending_projq.popleft()
                                    )
                                )
                for fn in deferred:
                    fn()
                while pending_av:
                    emit_av(*pending_av.popleft())

                # epilogue: O^T -> bf16, one batched DMA-transpose (src
                # partitions %16: pad 65->80; pad rows left uninitialized
                # — their transposed columns are never read), divide by
                # the sums column, store in two halves on two queues
                last = qp == TH // QPASS - 1
                osb = osbpool.tile([80, QPASS], PV_DT, tag="osb")
                ostrip = osbpool.tile([128, QPASS // 128 * D], PV_DT, tag="ostrip")
                if last:
                    # PE transposes + division straight from PSUM, in
                    # two pipelined halves: no DMA-transpose on the tail
                    # (the xbar path costs ~1.3us + 1.5us sem wait), and
                    # half 0's div/store overlaps half 1's transposes
                    tpo = stp.tile([128, QPASS], PV_DT, tag="st", name="tpf")
                    for hf in range(2):
                        nc.vector.tensor_copy(
                            osb[0 : D + 1, hf * NMM : (hf + 1) * NMM],
                            ot[:, hf * NMM : (hf + 1) * NMM],
                        )
                        for blk in range(4 * hf, 4 * hf + 4):
                            nc.tensor.transpose(
                                tpo[:, blk * 80 : blk * 80 + D + 1],
                                osb[0 : D + 1, blk * 128 : (blk + 1) * 128],
                                ident65[:],
                            )
                        tpo_v = tpo[:, hf * 320 : (hf + 1) * 320].rearrange(
                            "p (b m) -> p b m", m=80
                        )
                        rc4 = small.tile(
                            [128, 4], F32, tag="rc4", name=f"rc4_{hf}"
                        )
                        nc.vector.reciprocal(
                            rc4[:].rearrange("p (b m) -> p b m", m=1),
                            tpo_v[:, :, D : D + 1],
                        )
                        nc.vector.tensor_tensor(
                            ostrip[:, 4 * hf * D : (4 * hf + 4) * D].rearrange(
                                "p (b d) -> p b d", d=D
                            ),
                            tpo_v[:, :, 0:D],
                            rc4[:]
                            .rearrange("p (b m) -> p b m", m=1)
                            .broadcast_to([128, 4, D]),
                            mybir.AluOpType.mult,
                        )
                        (nc.sync, nc.scalar)[hf].dma_start(
                            out_d[
                                q0 + hf * NMM : q0 + (hf + 1) * NMM, :
                            ].rearrange("(b p) d -> p b d", p=128),
                            ostrip[:, 4 * hf * D : (4 * hf + 4) * D].rearrange(
                                "p (b d) -> p b d", d=D
                            ),
                        )
                else:
                    nc.vector.tensor_copy(osb[0 : D + 1, :], ot[:])
                    tpo = osbpool.tile(
                        [128, QPASS // 128 * 80], PV_DT, tag="tpo"
                    )
                    nc.sync.dma_start(
                        out=tpo[:].rearrange("p (b m) -> p b m", m=80),
                        in_=osb[:],
                        transpose=True,
                    )
                    tpo_v = tpo[:, 0 : QPASS // 128 * 80].rearrange(
                        "p (b m) -> p b m", m=80
                    )
                    rc8 = small.tile([128, QPASS // 128], F32, tag="rc8")
                    nc.vector.reciprocal(
                        rc8[:].rearrange("p (b m) -> p b m", m=1),
                        tpo_v[:, :, D : D + 1],
                    )
                    for hf in range(2):
                        nc.vector.tensor_tensor(
                            ostrip[:, 4 * hf * D : (4 * hf + 4) * D].rearrange(
                                "p (b d) -> p b d", d=D
                            ),
                            tpo_v[:, 4 * hf : 4 * hf + 4, 0:D],
                            rc8[:, 4 * hf : 4 * hf + 4]
                            .rearrange("p (b m) -> p b m", m=1)
                            .broadcast_to([128, 4, D]),
                            mybir.AluOpType.mult,
                        )
                        (nc.sync, nc.gpsimd)[hf].dma_start(
                            out_d[
                                q0 + hf * NMM : q0 + (hf + 1) * NMM, :
                            ].rearrange("(b p) d -> p b d", p=128),
                            ostrip[:, 4 * hf * D : (4 * hf + 4) * D].rearrange(
                                "p (b d) -> p b d", d=D
                            ),
                        )

    _elide_redundant_ldweights(nc)
    nc.compile()
    return nc


def _elide_redundant_ldweights(nc):
    """Drop an InstLdweights whose stationary AP is identical to the
    previous one with only plain matmuls between (the legalizer emits one
    load per matmul; consecutive same-weights loads are dead)."""
    removed = 0
    for blk in nc.main_func.blocks:
        last_key = {}  # row-group (base partition span) -> AP key
        keep = []
        for inst in blk.instructions:
            if isinstance(inst, mybir.InstLdweights):
                si = inst.sync_info
                clean = si is None or (not si.on_wait and not si.on_update)
                ap = inst.ins[0]
                key = repr(ap)
                bap = getattr(ap, "bass_ap", None)
                part0 = psz = None
                if bap is not None:
                    try:
                        part0 = bap.base_partition()
                        psz = bap.partition_size()
                    except Exception:
                        part0 = psz = None
                grp = (part0, psz)
                full = psz is None or part0 is None or psz > 64
                if clean and part0 is not None and last_key.get(grp) == key:
                    removed += 1
                    continue
                if full:
                    last_key.clear()
                    if part0 is not None:
                        last_key[grp] = key
                else:
                    # a load into one row-group leaves other groups intact
                    last_key = {
                        g: k
                        for g, k in last_key.items()
                        if g[0] + (g[1] or 128) <= part0
                        or part0 + (psz or 128) <= g[0]
                    }
                    last_key[grp] = key
                keep.append(inst)
                continue
            if getattr(inst, "engine", None) == mybir.EngineType.PE:
                if not (
                    isinstance(inst, mybir.InstMatmult)
                    and not getattr(inst, "is_transpose", False)
                ):
                    last_key = {}
            keep.append(inst)
        blk.instructions[:] = keep
    return removed


_NC_CACHE = None
LAST_RESULT = None


def _get_nc():
    global _NC_CACHE
    if _NC_CACHE is None:
        _NC_CACHE = _build_nc()
    return _NC_CACHE


def make_in_maps(x, Wq, Wk, Wv):
    x = np.asarray(x, dtype=np.float32)
    Wq = np.asarray(Wq, dtype=np.float32)
    Wk = np.asarray(Wk, dtype=np.float32)
    Wv = np.asarray(Wv, dtype=np.float32)
    wv8 = Wv / np.sqrt(np.float32(D))
    w3 = np.ascontiguousarray(
        np.concatenate([Wq, Wq, Wk, wv8, wv8, Wk], axis=1)
    ).astype(SCORE_NP)
    in_maps = []
    for c in range(NCORES):
        b, h = divmod(c, 2)
        xb = x[b]
        rot = np.concatenate([xb[h * TH : (h + 1) * TH], xb[(1 - h) * TH : (2 - h) * TH]])
        in_maps.append(
            {
                "xT": np.ascontiguousarray(rot.T).astype(SCORE_NP),
                "w3": w3,
            }
        )
    return in_maps


def run(in_maps, trace=False, **kwargs):
    global LAST_RESULT
    nc = _get_nc()
    LAST_RESULT = run_bass_kernel_spmd(
        nc, in_maps, core_ids=list(range(NCORES)), trace=trace, **kwargs
    )
    return LAST_RESULT


def assemble(results):
    out = np.empty((B, T, D), dtype=np.float32)
    for c in range(NCORES):
        b, h = divmod(c, 2)
        out[b, h * TH : (h + 1) * TH] = np.asarray(
            results[c]["out"], dtype=np.float32
        )
    return out


def kernel(x, Wq, Wk, Wv):
    res = run(make_in_maps(x, Wq, Wk, Wv), trace=bool(os.environ.get("BASS_TRACE")))
    return assemble(res.results)


# revision 14
# speedup vs baseline: 1.0519x; 1.0218x over previous
"""Single-head attention (B=4, T=4096, E=1024, D=64) on 8 TRN2 NeuronCores.

Sharding: data-parallel over (batch, query-half): core c -> batch c//2,
query half c%2.  Each core receives the full x[b] pre-transposed on the
host, with rows rotated so its OWN query half always occupies columns
0:2048 (keeps the SPMD graph identical across cores; attention is
permutation-invariant over keys).

Row-packed scores: key tiles are paired (8q+i, 8q+4+i) within each
1024-col quarter q.  k2p [128, 2048] holds K^T for the even chunk of a
quarter in rows 0:64 and the odd chunk in rows 64:128 (projection
stationaries [Wk|Wv] / [Wv|Wk] land K on the right partition half).
q2d [128, TH] holds Q^T duplicated to both halves via a [Wq|Wq]
stationary.  The two score matmuls of a pair run CONCURRENTLY on
disjoint PE row-groups, halving score time.

Head: x^T rides the sync HWDGE ring as FIVE ordered column-block
loads (512+512+1024*3 cols, all-e packed) so the first-needed columns
complete at full aggregate DMA rate ~9us in; weights ride the scalar
ring.  Pair-0 projections are emitted chunk-major (Q c0, KV c0, Q c1,
KV c1) so the first scores launch as soon as x cols 0:1024 land.

exp split: most score chunks go through the ACT table exp; a fixed
subset (pass0 pp%4==3, pass1 pp%3==2) runs on the otherwise-idle DVE
via the bf16 bit-trick  P = bitcast_bf16(int16(s*128*log2e + bias))
(~1.8% rms multiplicative noise, zero mean in log via magic bias),
rebalancing the ACT train from ~68us to ~49us busy.

V' = [V | ones] strips: quarters 0,1 via PE transpose (interleaved
into early pass-0 slots; a DMA-transpose would deadlock-guard-wait on
the x stream), quarters 2,3 via batched DMA-transpose.  The ones
column makes P @ V' emit softmax row sums.

Per 1024-query pass, per key-tile pair: concurrent S^T matmuls, exp
(ACT or DVE) PSUM -> SBUF bf16, O^T += V'.T @ P^T into a [65, 1024]
PSUM accumulator (1/sqrt(D) folded into Wv).  AV emission lags the
score train (thr ~7 early in pass 0, ramping down) so it never blocks
the in-order PE queue on a not-yet-transposed V' strip.

Epilogue: O^T -> bf16; pass 0 via one batched DMA-transpose, last
pass via PE transposes; one reciprocal over the sum columns +
broadcast multiplies; bf16 stores in two halves on two queues (host
upcasts to f32).

PSUM: stp 3x[128,1024] (6 banks) + ot [65,1024] (2 banks) = 8 banks.

Softmax runs without max-subtraction: scores are ~N(0, 64) so |s| << 88
(fp32 exp overflow); the reference's max-subtraction is a no-op.
"""

import os
import sys
from collections import deque

import numpy as np

_TRN_REPO = "/opt/trn_rl_repo"
if _TRN_REPO not in sys.path:
    sys.path.insert(0, _TRN_REPO)

import concourse.bass as bass  # noqa: E402
import concourse.mybir as mybir  # noqa: E402
import concourse.tile as tile  # noqa: E402
from concourse import bacc  # noqa: E402
from concourse.bass_utils import run_bass_kernel_spmd  # noqa: E402

F32 = mybir.dt.float32
F16 = mybir.dt.float16
BF16 = mybir.dt.bfloat16
I16 = mybir.dt.int16

B, T, E, D = 4, 4096, 1024, 64
TH = T // 2  # queries per core
NCORES = 8
QPASS = 1024  # queries per PSUM pass
NMM = 512  # matmul moving free dim (one fp32 PSUM bank)
NKT = T // 128  # 32 key tiles of 128
EK = E // 128  # 8 contraction tiles for projections
QW = T // 4  # x^T block width (1024)

SCORE_DT = F16
SCORE_NP = np.float16
PV_DT = BF16  # P = exp(S) reaches ~1e20: needs bf16 range

# DVE bit-trick exp: P ~= bitcast_bf16(int16(s * 128*log2e + BIAS)).
# BIAS = 127*128 - 7.33 makes the piecewise-linear mantissa error
# zero-mean in log space (so DVE-exp'd keys carry no systematic weight
# offset vs ACT-exp'd keys in the same softmax row).
EXP_SCALE = 128.0 / float(np.log(2.0))
EXP_BIAS = 127.0 * 128.0 - 7.33


def _dve_chunk(qp, pp, half):
    """Which exp chunks run on DVE instead of the ACT table.  Running
    the two halves of a pp on DIFFERENT engines frees both st PSUM
    tiles concurrently (3-buf rotation would otherwise stall the score
    matmuls 1.5 slots later on the serial ACT train)."""
    if qp == 0:
        return half == 1 and pp % 2 == 1
    return half == 1


def _build_nc() -> bass.Bass:
    nc = bacc.Bacc(
        "TRN2",
        target_bir_lowering=False,
        debug=False,
        num_devices=NCORES,
    )
    xT_d = nc.dram_tensor("xT", [E, T], SCORE_DT, kind="ExternalInput")
    # [Wq|Wq | Wk|Wv/8 | Wv/8|Wk] packed so ONE dma covers all weights
    w3_d = nc.dram_tensor("w3", [E, 384], SCORE_DT, kind="ExternalInput")
    out_d = nc.dram_tensor("out", [TH, D], PV_DT, kind="ExternalOutput")

    with tile.TileContext(nc) as tc:
        with (
            tc.tile_pool(name="consts", bufs=1) as consts,
            tc.tile_pool(name="big", bufs=1) as big,
            tc.tile_pool(name="pt", bufs=14) as ptpool,
            tc.tile_pool(name="osb", bufs=2) as osbpool,
            tc.tile_pool(name="small", bufs=6) as small,
            tc.tile_pool(name="stp", bufs=3, space="PSUM") as stp,
            tc.tile_pool(name="otp", bufs=1, space="PSUM") as otp,
        ):
            # ---- weights: one fat load on the scalar ring (splitting
            # by weight block halves the descriptor size to 256B,
            # sub-line-rate, and lands slower) ----
            w3 = consts.tile([128, EK * 384], SCORE_DT, tag="w3")
            nc.scalar.dma_start(
                w3[:].rearrange("p (e m) -> p e m", e=EK),
                w3_d.rearrange("(e p) m -> p e m", p=128),
            )

            def wqq(e):
                return w3[:, e * 384 : e * 384 + 128]

            def wkv(e):
                return w3[:, e * 384 + 128 : e * 384 + 256]

            def wvk(e):
                return w3[:, e * 384 + 256 : e * 384 + 384]

            # ---- x^T loads: five ordered column-block transfers on the
            # sync HWDGE ring (FIFO per ring; each InstDMACopy spreads
            # across all 16 SDMA engines, so block k completes before
            # block k+1 starts).  Chunks 0,1 ride alone so the first
            # projections can start ~9us in. ----
            xb0 = []
            for h in range(2):
                xt = big.tile([128, EK * NMM], SCORE_DT, tag=f"xb0{h}")
                nc.sync.dma_start(
                    xt[:].rearrange("p (e m) -> p e m", e=EK),
                    xT_d[:, h * NMM : (h + 1) * NMM].rearrange(
                        "(e p) m -> p e m", p=128
                    ),
                )
                xb0.append(xt)
            xblk = {}
            for b in (1, 2, 3):
                xt = big.tile([128, EK * QW], SCORE_DT, tag=f"xblk{b}")
                nc.sync.dma_start(
                    xt[:].rearrange("p (e m) -> p e m", e=EK),
                    xT_d[:, b * QW : (b + 1) * QW].rearrange(
                        "(e p) m -> p e m", p=128
                    ),
                )
                xblk[b] = xt

            def xt_ap(e, cg):
                # proj chunk cg covers x^T cols [cg*512, cg*512+512)
                if cg < 2:
                    return xb0[cg][:, e * NMM : (e + 1) * NMM]
                b, half = divmod(cg, 2)
                c0 = e * QW + half * NMM
                return xblk[b][:, c0 : c0 + NMM]

            # warm tile first: the HAM warmup matmuls must start the
            # moment the preamble ends, and DVE runs its queue in order
            warm = consts.tile([128, NMM], SCORE_DT, tag="warm")
            nc.vector.memset(warm[:], 0.0)
            # V' strip: 32 tiles of [128 keys, 64 V cols + 1 ones col],
            # padded to stride 128.  Only the ones columns need init.
            vprime = consts.tile([128, NKT * 128], PV_DT, tag="vprime")
            nc.vector.memset(
                vprime[:].rearrange("p (b m) -> p b m", m=128)[:, :, 64:65],
                1.0,
            )
            ident = consts.tile([128, 64], PV_DT, tag="ident")
            from concourse.masks import make_identity

            make_identity(nc, ident[0:64, :])
            make_identity(nc, ident[64:128, :])
            ident65 = consts.tile([65, 65], PV_DT, tag="ident65")
            make_identity(nc, ident65[:])
            # preload the exp table set (~2.7us) while DMAs stream
            pre = small.tile([128, 32], PV_DT, tag="pre")
            nc.scalar.activation(
                pre[:], warm[:, 0:32], mybir.ActivationFunctionType.Exp
            )
            # HAM warmup: keep PE busy (and the pstate ramp warm) until
            # the first x block lands
            wps = stp.tile([128, QPASS], F32, tag="st", name="wps")
            for _ in range(17):
                nc.tensor.matmul(
                    wps[:, 0:256], warm[:, 0:128], warm[:, 0:256],
                    start=True, stop=True,
                )

            q2d = big.tile([128, TH], SCORE_DT, tag="q2d")
            k2p = big.tile([128, T // 2], SCORE_DT, tag="k2p")
            vt = big.tile([128, T], PV_DT, tag="vt")

            # ---- projection pair emitter: pair = chunks (cg0, cg0+1)
            # covering quarter q=cg0//2.  K of the even chunk -> k2p
            # rows 0:64, odd chunk -> rows 64:128 (via [Wk|Wv]/[Wv|Wk]);
            # V^T to vt rows 64:128 / 0:64 respectively. ----
            proj_t = {}

            def emit_proj(cg0, part, sub):
                """One 4-MM slice of a projection pair.  part 0/1 =
                [Q|Q] sweep chunk cg0/cg0+1 (own pairs only); part 2/3 =
                K/V sweep ([Wk|Wv] even chunk / [Wv|Wk] odd).  sub=1
                finishes the chunk and emits its copies (+V' transpose
                for quarters 2,3)."""
                q = cg0 // 2
                kcol = q * NMM  # k2p column block for this quarter
                kb0 = 8 * q
                half = part & 1  # 0: even chunk cg0, 1: odd chunk cg0+1
                cg = cg0 + half
                sl = slice(half * NMM, (half + 1) * NMM)

                if part in (0, 1):  # [Q|Q] sweep halves (own pairs only)
                    if part == 0 and sub == 0:
                        proj_t[cg0] = stp.tile(
                            [128, QPASS], F32, tag="st", name=f"p1_{cg0}"
                        )
                    p1 = proj_t[cg0]
                    for e in range(4 * sub, 4 * sub + 4):
                        nc.tensor.matmul(
                            p1[:, sl],
                            wqq(e),
                            xt_ap(e, cg),
                            start=(e == 0),
                            stop=(e == EK - 1),
                        )
                    if sub == 1:
                        nc.vector.tensor_copy(
                            q2d[:, cg * NMM : (cg + 1) * NMM], p1[:, sl]
                        )
                    return

                # K/V sweep: wkv for the even chunk (K -> k2p rows 0:64,
                # V^T -> vt rows 64:128), wvk for the odd (swapped)
                if part == 2 and sub == 0:
                    proj_t[cg0 + 8] = stp.tile(
                        [128, QPASS], F32, tag="st", name=f"p2_{cg0}"
                    )
                p2 = proj_t[cg0 + 8]
                w = wkv if half == 0 else wvk
                for e in range(4 * sub, 4 * sub + 4):
                    nc.tensor.matmul(
                        p2[:, sl],
                        w(e),
                        xt_ap(e, cg),
                        start=(e == 0),
                        stop=(e == EK - 1),
                    )
                if sub == 0:
                    return
                vrow, krow = (64, 0) if half == 0 else (0, 64)
                # k2p first: the score matmuls block on it; vt only
                # feeds the V' transposes, which run later
                nc.vector.tensor_copy(
                    k2p[krow : krow + 64, kcol : kcol + NMM],
                    p2[krow : krow + 64, sl],
                )
                nc.vector.tensor_copy(
                    vt[vrow : vrow + 64, cg * NMM : (cg + 1) * NMM],
                    p2[vrow : vrow + 64, sl],
                )
                if cg0 >= 4:
                    nc.sync.dma_start(
                        out=vprime[
                            :, (kb0 + 4 * half) * 128 : (kb0 + 4 * half + 4) * 128
                        ].rearrange("p (b m) -> p b m", m=128)[:, :, 0:64],
                        in_=vt[vrow : vrow + 64, cg * NMM : (cg + 1) * NMM],
                        transpose=True,
                    )

            def emit_tp0(cg):
                # V' strips for chunk cg (quarters 0,1) via PE
                # transpose: a DMA-transpose would deadlock-guard-wait
                # on the whole x stream.  All 4 strips of the chunk in
                # one PSUM tile + one batched copy (fewer st-pool
                # rotations).
                vrow = 64 if cg % 2 == 0 else 0
                tps = stp.tile(
                    [128, QPASS], PV_DT, tag="st", name=f"tp0_{cg}"
                )
                for s in range(4):
                    nc.tensor.transpose(
                        tps[:, s * 64 : (s + 1) * 64],
                        vt[
                            vrow : vrow + 64,
                            cg * NMM + s * 128 : cg * NMM + (s + 1) * 128,
                        ],
                        ident[vrow : vrow + 64, :],
                    )
                nc.vector.tensor_copy(
                    vprime[
                        :, (4 * cg) * 128 : (4 * cg + 4) * 128
                    ].rearrange("p (b m) -> p b m", m=128)[:, :, 0:64],
                    tps[:, 0:256].rearrange("p (b m) -> p b m", m=64),
                )

            # ---- pair-0 projections, chunk-major so the first scores
            # only wait on x cols 0:1024: Q c0, KV c0, Q c1, KV c1 ----
            p1_0 = stp.tile([128, QPASS], F32, tag="st", name="p1_0")
            p2_0 = stp.tile([128, QPASS], F32, tag="st", name="p2_0")
            proj_t[0] = p1_0
            proj_t[8] = p2_0
            pair0_vt = []
            for e in range(EK):
                nc.tensor.matmul(
                    p1_0[:, 0:NMM], wqq(e), xt_ap(e, 0),
                    start=(e == 0), stop=(e == EK - 1),
                )
            nc.vector.tensor_copy(q2d[:, 0:NMM], p1_0[:, 0:NMM])
            for e in range(EK):
                nc.tensor.matmul(
                    p2_0[:, 0:NMM], wkv(e), xt_ap(e, 0),
                    start=(e == 0), stop=(e == EK - 1),
                )
            nc.vector.tensor_copy(k2p[0:64, 0:NMM], p2_0[0:64, 0:NMM])
            pair0_vt.append(
                lambda: nc.vector.tensor_copy(
                    vt[64:128, 0:NMM], p2_0[64:128, 0:NMM]
                )
            )
            for e in range(EK):
                nc.tensor.matmul(
                    p1_0[:, NMM:QPASS], wqq(e), xt_ap(e, 1),
                    start=(e == 0), stop=(e == EK - 1),
                )
            nc.vector.tensor_copy(q2d[:, NMM:QPASS], p1_0[:, NMM:QPASS])
            for e in range(EK):
                nc.tensor.matmul(
                    p2_0[:, NMM:QPASS], wvk(e), xt_ap(e, 1),
                    start=(e == 0), stop=(e == EK - 1),
                )
            nc.vector.tensor_copy(k2p[64:128, 0:NMM], p2_0[64:128, NMM:QPASS])
            # vt copies go AFTER both k2p copies on the DVE queue — the
            # first scores block on k2p; the V' strips run a bit later
            for fn in pair0_vt:
                fn()
            pair0_vt.clear()
            nc.vector.tensor_copy(vt[0:64, NMM : 2 * NMM], p2_0[0:64, NMM:QPASS])
            # quarter-0 V' strips in the head: PE would otherwise idle
            # here waiting on the k2p copies before the first scores
            emit_tp0(0)
            emit_tp0(1)

            # pass-0 interleave queues (one pop per half-slot; a popped
            # item is EMITTED one slot later, so a strip placed directly
            # after its chunk's KV sub-part is emitted strictly after
            # that part's vt copy — emission order IS the dependency
            # order under Tile).  proj2: quarters 2,3 KV (their V' goes
            # via DMA-transpose); projq: pair-2 Q sweep for pass 1.
            pending_proj = deque(
                [(2, 2, 0), (2, 2, 1), ("tp0", 2),
                 (2, 3, 0), (2, 3, 1), ("tp0", 3)]
            )
            pending_proj2 = deque(
                [(cg, p, s) for cg in (4, 6) for p in (2, 3) for s in range(2)]
            )
            pending_projq = deque([(2, p, s) for p in (0, 1) for s in range(2)])

            # ---- attention passes ----
            for qp in range(TH // QPASS):
                q0 = qp * QPASS
                ot = otp.tile([D + 1, QPASS], F32, tag="ot")
                pending_av = deque()

                def emit_av(avpt, avkt):
                    for qc in range(0, QPASS, NMM):
                        nc.tensor.matmul(
                            ot[:, qc : qc + NMM],
                            vprime[:, avkt * 128 : avkt * 128 + D + 1],
                            avpt[:, qc : qc + NMM],
                            start=(avkt == 0),
                            stop=(avkt == NKT - 1),
                        )

                deferred = []
                for pp in range(16):
                    # key-tile pair (8q+i, 8q+4+i): q = pp//4, i = pp%4
                    qq, i = divmod(pp, 4)
                    ktA = 8 * qq + i
                    ktB = ktA + 4
                    kc = qq * NMM + i * 128
                    sts = []
                    for half in (0, 1):
                        st = stp.tile(
                            [128, QPASS], F32, tag="st",
                            name=f"st{qp}_{pp}_{half}",
                        )
                        sts.append(st)
                    if qp == 0 and pp == 0:
                        # A-half first: the first exp unblocks earlier
                        mm_order = [(q, h) for h in (0, 1) for q in (0, NMM)]
                    else:
                        mm_order = [(q, h) for q in (0, NMM) for h in (0, 1)]
                    for qc, half in mm_order:
                        base = 64 * half
                        nc.tensor.matmul(
                            sts[half][:, qc : qc + NMM],
                            k2p[base : base + 64, kc : kc + 128],
                            q2d[base : base + 64, q0 + qc : q0 + qc + NMM],
                            start=True,
                            stop=True,
                        )
                    # previous slot's AV/projection PE work goes AFTER
                    # this slot's score matmuls: the exp train then only
                    # ever waits on the 4 ST matmuls at a block's head
                    for fn in deferred:
                        fn()
                    deferred = []
                    # AV backlog: deep early in pass 0 (V' strips and
                    # the x stream must stay ahead of the in-order PE
                    # queue), draining to 1 by the pass tail
                    thr = (
                        max(1, 7 - 2 * max(0, pp - 11))
                        if qp == 0
                        else (1 if pp == 15 else 2)
                    )
                    for half, kt in ((0, ktA), (1, ktB)):
                        pt = ptpool.tile(
                            [128, QPASS], PV_DT, tag="pt",
                            name=f"pt{qp}_{pp}_{half}",
                        )
                        if _dve_chunk(qp, pp, half):
                            nc.vector.tensor_scalar(
                                pt[:].bitcast(I16),
                                sts[half][:],
                                EXP_SCALE,
                                EXP_BIAS,
                                mybir.AluOpType.mult,
                                mybir.AluOpType.add,
                            )
                        else:
                            nc.scalar.activation(
                                pt[:], sts[half][:],
                                mybir.ActivationFunctionType.Exp,
                            )
                        pending_av.append((pt, kt))
                        while len(pending_av) > thr:
                            deferred.append(
                                (lambda a: lambda: emit_av(*a))(
                                    pending_av.popleft()
                                )
                            )
                        if qp == 0:
                            if pending_proj:
                                item = pending_proj.popleft()
                                if item[0] == "tp0":
                                    deferred.append(
                                        (lambda it: lambda: emit_tp0(it[1]))(
                                            item
                                        )
                                    )
                                else:
                                    deferred.append(
                                        (lambda it: lambda: emit_proj(*it))(
                                            item
                                        )
                                    )
                            elif pp >= 4 and pending_proj2:
                                deferred.append(
                                    (lambda it: lambda: emit_proj(*it))(
                                        pending_proj2.popleft()
                                    )
                                )
                            elif (
                                pp >= 10 and half == 0 and pending_projq
                            ):
                                deferred.append(
                                    (lambda it: lambda: emit_proj(*it))(
                                        pending_projq.popleft()
                                    )
                                )
                for fn in deferred:
                    fn()
                while pending_av:
                    emit_av(*pending_av.popleft())

                # epilogue: O^T -> bf16, one batched DMA-transpose (src
                # partitions %16: pad 65->80; pad rows left uninitialized
                # — their transposed columns are never read), divide by
                # the sums column, store in two halves on two queues
                last = qp == TH // QPASS - 1
                osb = osbpool.tile([80, QPASS], PV_DT, tag="osb")
                ostrip = osbpool.tile([128, QPASS // 128 * D], PV_DT, tag="ostrip")
                if last:
                    # PE transposes + division straight from PSUM, in
                    # two pipelined halves: no DMA-transpose on the tail
                    # (the xbar path costs ~1.3us + 1.5us sem wait), and
                    # half 0's div/store overlaps half 1's transposes
                    tpo = stp.tile([128, QPASS], PV_DT, tag="st", name="tpf")
                    for hf in range(2):
                        nc.vector.tensor_copy(
                            osb[0 : D + 1, hf * NMM : (hf + 1) * NMM],
                            ot[:, hf * NMM : (hf + 1) * NMM],
                        )
                        for blk in range(4 * hf, 4 * hf + 4):
                            nc.tensor.transpose(
                                tpo[:, blk * 80 : blk * 80 + D + 1],
                                osb[0 : D + 1, blk * 128 : (blk + 1) * 128],
                                ident65[:],
                            )
                        tpo_v = tpo[:, hf * 320 : (hf + 1) * 320].rearrange(
                            "p (b m) -> p b m", m=80
                        )
                        rc4 = small.tile(
                            [128, 4], F32, tag="rc4", name=f"rc4_{hf}"
                        )
                        nc.vector.reciprocal(
                            rc4[:].rearrange("p (b m) -> p b m", m=1),
                            tpo_v[:, :, D : D + 1],
                        )
                        nc.vector.tensor_tensor(
                            ostrip[:, 4 * hf * D : (4 * hf + 4) * D].rearrange(
                                "p (b d) -> p b d", d=D
                            ),
                            tpo_v[:, :, 0:D],
                            rc4[:]
                            .rearrange("p (b m) -> p b m", m=1)
                            .broadcast_to([128, 4, D]),
                            mybir.AluOpType.mult,
                        )
                        (nc.sync, nc.scalar)[hf].dma_start(
                            out_d[
                                q0 + hf * NMM : q0 + (hf + 1) * NMM, :
                            ].rearrange("(b p) d -> p b d", p=128),
                            ostrip[:, 4 * hf * D : (4 * hf + 4) * D].rearrange(
                                "p (b d) -> p b d", d=D
                            ),
                        )
                else:
                    nc.vector.tensor_copy(osb[0 : D + 1, :], ot[:])
                    tpo = osbpool.tile(
                        [128, QPASS // 128 * 80], PV_DT, tag="tpo"
                    )
                    nc.sync.dma_start(
                        out=tpo[:].rearrange("p (b m) -> p b m", m=80),
                        in_=osb[:],
                        transpose=True,
                    )
                    tpo_v = tpo[:, 0 : QPASS // 128 * 80].rearrange(
                        "p (b m) -> p b m", m=80
                    )
                    rc8 = small.tile([128, QPASS // 128], F32, tag="rc8")
                    nc.vector.reciprocal(
                        rc8[:].rearrange("p (b m) -> p b m", m=1),
                        tpo_v[:, :, D : D + 1],
                    )
                    for hf in range(2):
                        nc.vector.tensor_tensor(
                            ostrip[:, 4 * hf * D : (4 * hf + 4) * D].rearrange(
                                "p (b d) -> p b d", d=D
                            ),
                            tpo_v[:, 4 * hf : 4 * hf + 4, 0:D],
                            rc8[:, 4 * hf : 4 * hf + 4]
                            .rearrange("p (b m) -> p b m", m=1)
                            .broadcast_to([128, 4, D]),
                            mybir.AluOpType.mult,
                        )
                        (nc.sync, nc.gpsimd)[hf].dma_start(
                            out_d[
                                q0 + hf * NMM : q0 + (hf + 1) * NMM, :
                            ].rearrange("(b p) d -> p b d", p=128),
                            ostrip[:, 4 * hf * D : (4 * hf + 4) * D].rearrange(
                                "p (b d) -> p b d", d=D
                            ),
                        )

    _elide_redundant_ldweights(nc)
    nc.compile()
    return nc


def _elide_redundant_ldweights(nc):
    """Drop an InstLdweights whose stationary AP is identical to the
    previous one with only plain matmuls between (the legalizer emits one
    load per matmul; consecutive same-weights loads are dead)."""
    removed = 0
    for blk in nc.main_func.blocks:
        last_key = {}  # row-group (base partition span) -> AP key
        keep = []
        for inst in blk.instructions:
            if isinstance(inst, mybir.InstLdweights):
                si = inst.sync_info
                clean = si is None or (not si.on_wait and not si.on_update)
                ap = inst.ins[0]
                key = repr(ap)
                bap = getattr(ap, "bass_ap", None)
                part0 = psz = None
                if bap is not None:
                    try:
                        part0 = bap.base_partition()
                        psz = bap.partition_size()
                    except Exception:
                        part0 = psz = None
                grp = (part0, psz)
                full = psz is None or part0 is None or psz > 64
                if clean and part0 is not None and last_key.get(grp) == key:
                    removed += 1
                    continue
                if full:
                    last_key.clear()
                    if part0 is not None:
                        last_key[grp] = key
                else:
                    # a load into one row-group leaves other groups intact
                    last_key = {
                        g: k
                        for g, k in last_key.items()
                        if g[0] + (g[1] or 128) <= part0
                        or part0 + (psz or 128) <= g[0]
                    }
                    last_key[grp] = key
                keep.append(inst)
                continue
            if getattr(inst, "engine", None) == mybir.EngineType.PE:
                if not (
                    isinstance(inst, mybir.InstMatmult)
                    and not getattr(inst, "is_transpose", False)
                ):
                    last_key = {}
            keep.append(inst)
        blk.instructions[:] = keep
    return removed


_NC_CACHE = None
LAST_RESULT = None


def _get_nc():
    global _NC_CACHE
    if _NC_CACHE is None:
        _NC_CACHE = _build_nc()
    return _NC_CACHE


def make_in_maps(x, Wq, Wk, Wv):
    x = np.asarray(x, dtype=np.float32)
    Wq = np.asarray(Wq, dtype=np.float32)
    Wk = np.asarray(Wk, dtype=np.float32)
    Wv = np.asarray(Wv, dtype=np.float32)
    wv8 = Wv / np.sqrt(np.float32(D))
    w3 = np.ascontiguousarray(
        np.concatenate([Wq, Wq, Wk, wv8, wv8, Wk], axis=1)
    ).astype(SCORE_NP)
    in_maps = []
    for c in range(NCORES):
        b, h = divmod(c, 2)
        xb = x[b]
        rot = np.concatenate([xb[h * TH : (h + 1) * TH], xb[(1 - h) * TH : (2 - h) * TH]])
        in_maps.append(
            {
                "xT": np.ascontiguousarray(rot.T).astype(SCORE_NP),
                "w3": w3,
            }
        )
    return in_maps


def run(in_maps, trace=False, **kwargs):
    global LAST_RESULT
    nc = _get_nc()
    LAST_RESULT = run_bass_kernel_spmd(
        nc, in_maps, core_ids=list(range(NCORES)), trace=trace, **kwargs
    )
    return LAST_RESULT


def assemble(results):
    out = np.empty((B, T, D), dtype=np.float32)
    for c in range(NCORES):
        b, h = divmod(c, 2)
        out[b, h * TH : (h + 1) * TH] = np.asarray(
            results[c]["out"], dtype=np.float32
        )
    return out


def kernel(x, Wq, Wk, Wv):
    res = run(make_in_maps(x, Wq, Wk, Wv), trace=bool(os.environ.get("BASS_TRACE")))
    return assemble(res.results)
